# revision 48
# baseline (speedup 1.0000x reference)
"""CrossAttentionBlock3D on 8 TRN2 NeuronCores — sequence-parallel Bass kernel.

Sharding: the 32768 spatial tokens are split 8x4096 across cores. GroupNorm
statistics are the only cross-core dependency (one 64-float AllGather).
Everything else (LN, K/V projections over the tiny context, Q/attention/proj
for the local tokens) is computed locally; context-side work is replicated.

v2: GroupNorm folded into effective Q weights (no h materialization), x kept
resident in SBUF, interleaved emission to keep the PE warm (q of iter i+1 and
proj of iter i-1 fill the softmax-exp gaps), z head-pairs row-tiled on the PE
array, batched rowsum DMA + exp(-ln(x)) reciprocal, single activation table
set, bf16 output.
"""
import sys

sys.path.insert(0, "/opt/trn_rl_repo")

import numpy as np
import ml_dtypes

from concourse import bass, bacc, tile, mybir, masks
from concourse.bass_utils import run_bass_kernel_spmd

F32 = mybir.dt.float32
BF16 = mybir.dt.bfloat16
BF = ml_dtypes.bfloat16
AF = mybir.ActivationFunctionType
ALU = mybir.AluOpType

B, C, D, H, W = 2, 512, 32, 32, 32
S = D * H * W              # 32768
L, CTX = 256, 768
NH, HD, G = 8, 64, 8
EPS = 1e-5
NCORES = 8
CT, OT = 4, 4              # channel tiles (C = 4*128)
KCT = 6                    # ctx channel tiles (CTX = 6*128)
LT = 2                     # L = 2*128
VP = 80                    # padded per-(head,chunk) v stride: 64 v + 1 ones + pad


def build(nc, s_loc, st):
    """Emit the per-core Tile program. s_loc: local tokens; st: S-tile."""
    nst = s_loc // st
    n_gn = (C // G) * S    # global elems per (b, g) group

    WCOLS = 2 * CT * C + 2 * KCT * C          # qw | kw | vw | pw packed
    BCOLS = 3 * CT + 2 * B * CT               # qb | kb | pb | gnw8 | gnb8
    x_in = nc.dram_tensor("x", [B, CT, 128, s_loc], BF16, kind="ExternalInput")
    ctx_in = nc.dram_tensor("ctx", [128, B * LT * CTX], F32, kind="ExternalInput")
    w_in = nc.dram_tensor("wpack", [128, WCOLS], BF16, kind="ExternalInput")
    b_in = nc.dram_tensor("bpack", [128, BCOLS], F32, kind="ExternalInput")
    sel_in = nc.dram_tensor("selp", [8, 4 * 128], BF16, kind="ExternalInput")
    rm_in = nc.dram_tensor("rmask", [32, 4], F32, kind="ExternalInput")
    bc2_in = nc.dram_tensor("bc2", [2, 128], F32, kind="ExternalInput")
    out_ext = nc.dram_tensor("out", [B, nst, 128, CT * st], F32,
                             kind="ExternalOutput")

    from contextlib import ExitStack
    with tile.TileContext(nc) as tc, ExitStack() as es:
        wp = es.enter_context(tc.tile_pool(name="wp", bufs=1))
        dram = es.enter_context(tc.tile_pool(name="dram", bufs=1, space="DRAM"))

        # ---- persistent SBUF tensors ----
        w_all = wp.tile([128, WCOLS], BF16, tag="wpack")
        b_all = wp.tile([128, BCOLS], F32, tag="bpack")
        qw_t = w_all[:, 0:CT * C]
        kw_t = w_all[:, CT * C:CT * C + KCT * C]
        vw_t = w_all[:, CT * C + KCT * C:CT * C + 2 * KCT * C]
        pw_t = w_all[:, CT * C + 2 * KCT * C:WCOLS]
        qb_t = b_all[:, 0:CT]
        kb_t = b_all[:, CT:2 * CT]
        pb_t = b_all[:, 2 * CT:3 * CT]
        gnw_t = b_all[:, 3 * CT:3 * CT + B * CT]
        gnb_t = b_all[:, 3 * CT + B * CT:BCOLS]

        qw_eff = wp.tile([128, B * CT * C], BF16, tag="qw_eff")
        qb_tot = wp.tile([128, B * CT], F32, tag="qb_tot")
        x_all = wp.tile([128, B * CT * s_loc], BF16, tag="x_all")
        kT_all = wp.tile([128, B * CT * L], BF16, tag="kT")
        VW = NH * (HD + 1)
        v_all = wp.tile([128, B * LT * VW], BF16, tag="v")
        sel_t = wp.tile([8, 4 * 128], BF16, tag="selp")
        ident = wp.tile([128, 128], BF16, tag="ident")
        stats_s = wp.tile([128, 16], F32, tag="stats")
        a_pc = wp.tile([128, B * CT], F32, tag="a_pc")
        bias_pc = wp.tile([128, B * CT], BF16, tag="bias_pc")

        def xs(b, ct, lo=0, size=None):
            base = (b * CT + ct) * s_loc + lo
            return x_all[:, base:base + (size if size is not None else s_loc)]

        # ---- setup ----
        with tc.tile_pool(name="setup", bufs=1) as sp, \
             tc.tile_pool(name="setup_ps", bufs=2, space="PSUM") as spp:
            rm_t = sp.tile([32, 4], F32, tag="rm")
            bc2_t = sp.tile([2, 128], F32, tag="bc2")
            masks.make_identity(nc, ident[:])
            ctxf = sp.tile([128, B * LT * CTX], F32, tag="ctxf")
            eps2 = sp.tile([2, 1], F32, tag="eps2")
            nc.vector.memset(eps2[:], EPS)
            eps128 = sp.tile([128, 1], F32, tag="eps128")
            nc.vector.memset(eps128[:], EPS)

            # x loads (2 DMA queues) + GN partial stats
            # (sum in stats cols 0:8, sumsq in 8:16)
            sinkS = sp.tile([128, s_loc], BF16, tag="sinkS")

            def load_stats(b):
                for ct in range(CT):
                    col = b * CT + ct
                    nc.gpsimd.dma_start(xs(b, ct), x_in[b, ct]) if b == 0 \
                        else nc.sync.dma_start(xs(b, ct), x_in[b, ct])
                    nc.vector.tensor_reduce(
                        stats_s[:, col:col + 1], xs(b, ct),
                        mybir.AxisListType.X, ALU.add)
                    nc.scalar.activation(
                        sinkS[:], xs(b, ct), AF.Square,
                        accum_out=stats_s[:, 8 + col:9 + col])

            # x on the gpsimd queue (b0) so the sync queue serves ctx/weights
            # immediately; the ctx/K/V path below keeps the PE busy during
            # the x loads and the GN AllGather round-trip.
            nc.sync.dma_start(ctxf[:], ctx_in[:])
            nc.sync.dma_start(w_all[:], w_in[:])
            nc.sync.dma_start(b_all[:], b_in[:])
            nc.sync.dma_start(sel_t[:], sel_in[:])
            nc.sync.dma_start(rm_t[:], rm_in[:])
            nc.sync.dma_start(bc2_t[:], bc2_in[:])

            mask2 = sp.tile([128, 2], F32, tag="mask2")
            nc.vector.memset(mask2[:, :], 0.0)
            nc.vector.memset(mask2[0:64, 0:1], 1.0)
            nc.vector.memset(mask2[64:128, 1:2], 1.0)

            # per-batch: AllGather partial stats, derive qw_eff/qb_tot
            def ag_dispatch(b):
                cs = slice(b * CT, (b + 1) * CT)
                st_p = spp.tile([4, 4], F32, tag="st_p", bufs=1,
                                name=f"st_p_{b}")
                nc.tensor.matmul(st_p[:, 0:2], stats_s[:, b * CT:(b + 1) * CT],
                                 mask2[:], start=True, stop=True)
                nc.tensor.matmul(st_p[:, 2:4],
                                 stats_s[:, 8 + b * CT:8 + (b + 1) * CT],
                                 mask2[:], start=True, stop=True)
                red_s = sp.tile([4, 4], F32, tag="red_s", bufs=2,
                                name=f"red_s_{b}")
                nc.vector.tensor_copy(red_s[:], st_p[:])

                cc_in = dram.tile([4, 4], F32, tag="cc_in", bufs=2,
                                  name=f"cc_in_{b}")
                cc_ag = dram.tile([32, 4], F32, tag="cc_ag", bufs=2,
                                  name=f"cc_ag_{b}")
                nc.gpsimd.dma_start(cc_in[:], red_s[:])
                nc.gpsimd.collective_compute(
                    "AllGather", mybir.AluOpType.bypass,
                    replica_groups=[list(range(NCORES))],
                    ins=[cc_in.opt()], outs=[cc_ag.opt()])
                ag_s = sp.tile([32, 4], F32, tag="ag_s", bufs=2,
                               name=f"ag_s_{b}")
                nc.gpsimd.dma_start(ag_s[:], cc_ag[:])
                return ag_s

            def ag_post(b, ag_s):
                cs = slice(b * CT, (b + 1) * CT)
                ms_p = spp.tile([2, 8], F32, tag="ms_p", bufs=1,
                                name=f"ms_p_{b}")
                nc.tensor.matmul(ms_p[:, 0:4], ag_s[:, 0:2], rm_t[:],
                                 start=True, stop=True)
                nc.tensor.matmul(ms_p[:, 4:8], ag_s[:, 2:4], rm_t[:],
                                 start=True, stop=True)

                # mu | rstd  into one [2, 8] sbuf tile for one broadcast MM
                mr8 = sp.tile([2, 8], F32, tag="mr8", bufs=2, name=f"mr8_{b}")
                var8 = sp.tile([2, 4], F32, tag="var8", bufs=2,
                               name=f"var8_{b}")
                sd8 = sp.tile([2, 4], F32, tag="sd8", bufs=2, name=f"sd8_{b}")
                nc.vector.tensor_scalar_mul(mr8[:, 0:4], ms_p[:, 0:4],
                                            1.0 / n_gn)
                nc.vector.tensor_scalar_mul(var8[:], ms_p[:, 4:8], 1.0 / n_gn)
                # var = E[x^2] - mu^2
                tmu2 = sp.tile([2, 4], F32, tag="tmu2", bufs=2,
                               name=f"tmu2_{b}")
                nc.vector.tensor_mul(tmu2[:], mr8[:, 0:4], mr8[:, 0:4])
                nc.vector.tensor_sub(var8[:], var8[:], tmu2[:])
                # rstd = 1 / sqrt(var + eps)
                nc.scalar.activation(sd8[:], var8[:], AF.Sqrt, bias=eps2[:])
                nc.vector.reciprocal(mr8[:, 4:8], sd8[:])

                mr_d = dram.tile([2, 8], F32, tag="mr_d", bufs=2,
                                 name=f"mr_d_{b}")
                nc.gpsimd.dma_start(mr_d[:], mr8[:])
                br_s = sp.tile([128, 8], F32, tag="br_s", bufs=2,
                               name=f"br_s_{b}")
                nc.gpsimd.dma_start(
                    br_s[:], mr_d[:, :].unsqueeze(1).broadcast_to((2, 64, 8)))
                # a = rstd * gn_w ; bias = gn_b - mu * a   (per channel)
                nc.vector.tensor_mul(a_pc[:, cs], br_s[:, 4:8], gnw_t[:, cs])
                tmp_bc = sp.tile([128, 4], F32, tag="tmp_bc", bufs=2,
                                 name=f"tmp_bc_{b}")
                nc.vector.tensor_mul(tmp_bc[:], br_s[:, 0:4], a_pc[:, cs])
                nc.vector.tensor_sub(bias_pc[:, cs], gnb_t[:, cs], tmp_bc[:])

                # qw_eff[b] = qw * a  (per input-channel scale)
                for ct in range(CT):
                    col = b * CT + ct
                    nc.vector.tensor_scalar(
                        qw_eff[:, col * C:(col + 1) * C],
                        qw_t[:, ct * C:(ct + 1) * C],
                        a_pc[:, col:col + 1], None, ALU.mult)
                # qb_tot[b] = qw.T @ bias + qb   (bias zero-padded to N=4)
                bias_pad = sp.tile([128, CT * 4], BF16, tag="bias_pad",
                                   bufs=2, name=f"bias_pad_{b}")
                nc.vector.memset(bias_pad[:], 0.0)
                for ct in range(CT):
                    nc.vector.tensor_copy(
                        bias_pad[:, ct * 4:ct * 4 + 1],
                        bias_pc[:, b * CT + ct:b * CT + ct + 1])
                for ot in range(OT):
                    qe_p = spp.tile([128, 4], F32, tag="qe_p", bufs=1,
                                    name=f"qe_p_{b}_{ot}")
                    for ct in range(CT):
                        nc.tensor.matmul(
                            qe_p[:],
                            qw_t[:, ct * C + ot * 128:ct * C + (ot + 1) * 128],
                            bias_pad[:, ct * 4:(ct + 1) * 4],
                            start=(ct == 0), stop=(ct == CT - 1))
                    nc.vector.tensor_add(qb_tot[:, b * CT + ot:b * CT + ot + 1],
                                         qe_p[:, 0:1], qb_t[:, ot:ot + 1])

            # ---- context path: LN + transpose + K/V ----
            # LN stats for all 4 (b, lt) slices batched into [128, 4] so the
            # DVE<->Scalar ping-pong happens once, not four times (the first
            # ctx transpose gates the whole K/V path).
            ctxn = sp.tile([128, B * LT * CTX], BF16, tag="ctxn")
            sinkC = sp.tile([128, CTX], F32, tag="sinkC")
            NLT = B * LT
            cs1 = sp.tile([128, NLT], F32, tag="cs1")
            cs2 = sp.tile([128, NLT], F32, tag="cs2")
            cmu = sp.tile([128, NLT], F32, tag="cmu")
            cex2 = sp.tile([128, NLT], F32, tag="cex2")
            cvar = sp.tile([128, NLT], F32, tag="cvar")
            csd = sp.tile([128, NLT], F32, tag="csd")
            crstd = sp.tile([128, NLT], F32, tag="crstd")
            cnm = sp.tile([128, NLT], F32, tag="cnm")
            for j in range(NLT):
                cv = ctxf[:, j * CTX:(j + 1) * CTX]
                nc.vector.tensor_reduce(cs1[:, j:j + 1], cv,
                                        mybir.AxisListType.X, ALU.add)
                nc.scalar.activation(sinkC[:], cv, AF.Square,
                                     accum_out=cs2[:, j:j + 1])
            nc.vector.tensor_scalar_mul(cmu[:], cs1[:], 1.0 / CTX)
            nc.vector.tensor_scalar_mul(cex2[:], cs2[:], 1.0 / CTX)
            nc.vector.scalar_tensor_tensor(
                cvar[:], cmu[:], -1.0, cmu[:], ALU.mult, ALU.mult)
            nc.vector.tensor_add(cvar[:], cvar[:], cex2[:])
            nc.scalar.activation(csd[:], cvar[:], AF.Sqrt, bias=eps128[:])
            nc.vector.reciprocal(crstd[:], csd[:])
            nc.vector.scalar_tensor_tensor(
                cnm[:], cmu[:], -1.0, crstd[:], ALU.mult, ALU.mult)
            for j in range(NLT):
                nc.vector.tensor_scalar(
                    ctxn[:, j * CTX:(j + 1) * CTX],
                    ctxf[:, j * CTX:(j + 1) * CTX],
                    crstd[:, j:j + 1], cnm[:, j:j + 1],
                    ALU.mult, ALU.add)

            # transpose ctxn -> ctxT_all  [128ctx, L] per (b, kct)
            ctxT_all = sp.tile([128, B * KCT * L], BF16, tag="ctxT")
            for b in range(B):
                for lt in range(LT):
                    for ct in range(KCT):
                        tp_p = spp.tile([128, 128], BF16, tag="tp_p")
                        nc.tensor.transpose(
                            tp_p[:],
                            ctxn[:, (b * LT + lt) * CTX + ct * 128:
                                 (b * LT + lt) * CTX + (ct + 1) * 128],
                            ident[:])
                        nc.scalar.copy(
                            ctxT_all[:, (b * KCT + ct) * L + lt * 128:
                                     (b * KCT + ct) * L + (lt + 1) * 128],
                            tp_p[:])

            # kT[b, ot] [128, L]
            for b in range(B):
                for ot in range(OT):
                    k_p = spp.tile([128, L], F32, tag="k_p", bufs=1)
                    for ct in range(KCT):
                        nc.tensor.matmul(
                            k_p[:],
                            kw_t[:, ct * C + ot * 128:ct * C + (ot + 1) * 128],
                            ctxT_all[:, (b * KCT + ct) * L:(b * KCT + ct + 1) * L],
                            start=(ct == 0), stop=(ct == KCT - 1))
                    nc.scalar.activation(
                        kT_all[:, (b * CT + ot) * L:(b * CT + ot + 1) * L],
                        k_p[:], AF.Identity, bias=kb_t[:, ot:ot + 1])

            # v'[b, lt] [128, NH*(HD+1)]  (per-head ones column appended)
            for b in range(B):
                for lt in range(LT):
                    v_p = spp.tile([128, C], F32, tag="v_p", bufs=1)
                    for ct in range(KCT):
                        nc.tensor.matmul(
                            v_p[:],
                            ctxT_all[:, (b * KCT + ct) * L + lt * 128:
                                     (b * KCT + ct) * L + (lt + 1) * 128],
                            vw_t[:, ct * C:(ct + 1) * C],
                            start=(ct == 0), stop=(ct == KCT - 1))
                    vs = v_all[:, (b * LT + lt) * VW:(b * LT + lt + 1) * VW]
                    nc.scalar.copy(
                        vs.rearrange("p (h e) -> p h e", e=HD + 1)[:, :, 0:HD],
                        v_p[:])
                    nc.vector.memset(
                        vs.rearrange("p (h e) -> p h e", e=HD + 1)
                        [:, :, HD:HD + 1], 1.0)

            # GN stats + AllGathers last in emission order: b1's stats
            # reduces fill b0's collective round-trip, and the PE spends
            # the wait on the ctx/K/V work above.
            load_stats(0)
            ag0 = ag_dispatch(0)
            load_stats(1)
            ag_post(0, ag0)
            ag1 = ag_dispatch(1)
            ag_post(1, ag1)

        # ---- main attention loop (software-pipelined) ----
        with tc.tile_pool(name="mp", bufs=2) as mp, \
             tc.tile_pool(name="op", bufs=3) as op, \
             tc.tile_pool(name="mm_ps", bufs=2, space="PSUM") as mmp, \
             tc.tile_pool(name="z_ps", bufs=1, space="PSUM") as zp, \
             tc.tile_pool(name="o_ps", bufs=1, space="PSUM") as opp, \
             tc.tile_pool(name="rs_dram", bufs=2, space="DRAM") as rsd:

            iters = [(b, sti) for b in range(B) for sti in range(nst)]
            NIT = len(iters)

            def emit_q(i):
                b, sti = iters[i]
                lo = sti * st
                q_s = mp.tile([128, CT * st], BF16, tag="q_s",
                              name=f"q_s_{i}")
                for ot in range(OT):
                    q_p = mmp.tile([128, st], F32, tag="mm_p",
                                   name=f"q_p_{i}_{ot}")
                    for ct in range(CT):
                        nc.tensor.matmul(
                            q_p[:],
                            qw_eff[:, (b * CT + ct) * C + ot * 128:
                                   (b * CT + ct) * C + (ot + 1) * 128],
                            xs(b, ct, lo, st),
                            start=(ct == 0), stop=(ct == CT - 1))
                    nc.scalar.activation(
                        q_s[:, ot * st:(ot + 1) * st], q_p[:],
                        AF.Identity,
                        bias=qb_tot[:, b * CT + ot:b * CT + ot + 1])
                return q_s

            def emit_z_pair(i, k, q_s):
                """scores+exp for heads (2k, 2k+1). The two heads' z MMs
                alternate base partition 0/64 back-to-back, so consecutive
                MMs land in disjoint PE row groups and overlap."""
                b, sti = iters[i]
                zts, ps = [], []
                for j in range(2):
                    zts.append(zp.tile([128, 2 * st], F32, tag="z_t", bufs=2,
                                       name=f"z_{i}_{2 * k + j}"))
                for c in range(LT):
                    for j in range(2):
                        po = j * 64
                        nc.tensor.matmul(
                            zts[j][:, c * st:(c + 1) * st],
                            kT_all[po:po + 64,
                                   (b * CT + k) * L + c * 128:
                                   (b * CT + k) * L + (c + 1) * 128],
                            q_s[po:po + 64, k * st:(k + 1) * st],
                            start=True, stop=True)
                for j in range(2):
                    p_t = mp.tile([128, 2 * st], BF16, tag="p_t", bufs=4,
                                  name=f"p_{i}_{2 * k + j}")
                    nc.scalar.activation(p_t[:], zts[j][:], AF.Exp)
                    ps.append(p_t)
                return ps

            def emit_o_head(i, hh, p_t, o_all):
                b, sti = iters[i]
                o_p = opp.tile([65, st], F32, tag="o_p", bufs=2,
                               name=f"o_{i}_{hh}")
                for c in range(LT):
                    vb = (b * LT + c) * VW + hh * (HD + 1)
                    nc.tensor.matmul(
                        o_p[:],
                        v_all[:, vb:vb + HD + 1],
                        p_t[:, c * st:(c + 1) * st],
                        start=(c == 0), stop=(c == LT - 1))
                dst = o_all[:, hh * st:(hh + 1) * st]
                if hh % 2 == 0:
                    nc.scalar.copy(dst, o_p[:])
                else:
                    nc.vector.tensor_copy(dst, o_p[:])

            def emit_rs(i, o_all):
                rs_d = rsd.tile([8, st], F32, tag="rs_d",
                                name=f"rs_d_{i}")
                for h in range(NH):
                    nc.gpsimd.dma_start(rs_d[h:h + 1, :],
                                        o_all[64:65, h * st:(h + 1) * st])
                rs8 = mp.tile([8, st], F32, tag="rs8", name=f"rs8_{i}")
                nc.gpsimd.dma_start(rs8[:], rs_d[:])
                rec_f = mp.tile([8, st], F32, tag="rec_f", name=f"recf_{i}")
                nc.vector.reciprocal_approx_fast(rec_f[:], rs8[:])
                rec = mp.tile([8, st], BF16, tag="rec", name=f"rec_{i}")
                nc.vector.tensor_copy(rec[:], rec_f[:])
                # broadcast rec to head-pair layout [128, 4*st] via DMA
                # (keeps the normalize chain off the PE and the PSUM pools)
                rec_d = rsd.tile([8, st], BF16, tag="rec_d",
                                 name=f"rec_d_{i}")
                nc.gpsimd.dma_start(rec_d[:], rec[:])
                rb = mp.tile([64, NH * st], BF16, tag="rec_bc",
                             name=f"rb_{i}")
                for hh in range(NH):
                    nc.sync.dma_start(
                        rb[:, hh * st:(hh + 1) * st],
                        rec_d[hh:hh + 1, :].unsqueeze(1)
                        .broadcast_to((1, 64, st)))
                return rb

            def emit_norm(i, o_all, rb):
                proj_rhs = mp.tile([128, CT * st], BF16, tag="proj_rhs",
                                   name=f"prhs_{i}")
                for ko in range(4):
                    for j in range(2):
                        hh = 2 * ko + j
                        po = j * 64
                        nc.vector.tensor_tensor(
                            proj_rhs[po:po + 64, ko * st:(ko + 1) * st],
                            o_all[0:64, hh * st:(hh + 1) * st],
                            rb[:, hh * st:(hh + 1) * st], ALU.mult)
                return proj_rhs

            def emit_proj(i, proj_rhs, half):
                b, sti = iters[i]
                lo = sti * st
                if half == 0:
                    emit_proj.out_j = op.tile([128, CT * st], F32,
                                              tag="out_j", name=f"out_{i}")
                out_j = emit_proj.out_j
                for ot in (0, 1) if half == 0 else (2, 3):
                    y_p = mmp.tile([128, st], F32, tag="mm_p",
                                   name=f"y_p_{i}_{ot}")
                    for ct in range(CT):
                        nc.tensor.matmul(
                            y_p[:],
                            pw_t[:, ct * C + ot * 128:ct * C + (ot + 1) * 128],
                            proj_rhs[:, ct * st:(ct + 1) * st],
                            start=(ct == 0), stop=(ct == CT - 1))
                    nc.vector.scalar_tensor_tensor(
                        out_j[:, ot * st:(ot + 1) * st], y_p[:],
                        pb_t[:, ot:ot + 1], xs(b, ot, lo, st),
                        ALU.add, ALU.add)
                if half == 1:
                    nc.sync.dma_start(out_ext[b, sti], out_j[:])

            # software-pipelined emission: iter i-1's rowsum roundtrip,
            # normalize and proj are sandwiched between iter i's per-head
            # z/o chains so the PE always has independent work during the
            # softmax exp waits (keeps HAM from re-throttling the PE).
            q_cur = emit_q(0)
            prev = None            # (i, o_all)
            for i in range(NIT):
                o_all = mp.tile([65, NH * st], F32, tag="o_all",
                                name=f"o_all_{i}")
                if prev is not None:
                    rec_p = emit_rs(prev[0], prev[1])
                for k in range(4):
                    ps = emit_z_pair(i, k, q_cur)
                    emit_o_head(i, 2 * k, ps[0], o_all)
                    emit_o_head(i, 2 * k + 1, ps[1], o_all)
                q_next = emit_q(i + 1) if i + 1 < NIT else None
                if prev is not None:
                    prhs = emit_norm(prev[0], prev[1], rec_p)
                    emit_proj(prev[0], prhs, 0)
                    emit_proj(prev[0], prhs, 1)
                prev = (i, o_all)
                q_cur = q_next
            rec_p = emit_rs(prev[0], prev[1])
            prhs = emit_norm(prev[0], prev[1], rec_p)
            emit_proj(prev[0], prhs, 0)
            emit_proj(prev[0], prhs, 1)
    return nc


def prep_inputs(x, context, gn_w, gn_b, ln_w, ln_b, q_w, q_b, k_w, k_b,
                v_w, v_b, proj_w, proj_b, s_loc):
    """Host-side shard + layout prep. Returns in_maps for the 8 cores."""
    scale = HD ** -0.5
    qwT = (q_w.astype(np.float64) * scale).T.astype(np.float32)
    kwT = (k_w.astype(np.float64) * ln_w.astype(np.float64)[None, :]).T.astype(np.float32)
    vwT = (v_w.astype(np.float64) * ln_w.astype(np.float64)[None, :]).T.astype(np.float32)
    pwT = proj_w.T.astype(np.float32)
    kb_eff = (k_b + ln_b @ k_w.T).astype(np.float32)
    vb_eff = (v_b + ln_b @ v_w.T).astype(np.float32)
    pb_eff = (proj_b + vb_eff @ proj_w.T).astype(np.float32)
    qb_eff = (q_b * scale).astype(np.float32)

    gnw8 = np.empty((128, B * CT), np.float32)
    gnb8 = np.empty((128, B * CT), np.float32)
    for b in range(B):
        for t in range(CT):
            gnw8[:, b * CT + t] = gn_w[t * 128:(t + 1) * 128]
            gnb8[:, b * CT + t] = gn_b[t * 128:(t + 1) * 128]

    # pair-select: selp[r, ko*128 + m] = (r == 2*ko + (m >= 64))
    selp = np.zeros((8, 4 * 128), np.float32)
    for ko in range(4):
        selp[2 * ko, ko * 128:ko * 128 + 64] = 1.0
        selp[2 * ko + 1, ko * 128 + 64:(ko + 1) * 128] = 1.0
    rmask = (np.arange(32)[:, None] % 4 == np.arange(4)[None, :]).astype(np.float32)
    bc2 = (np.arange(128)[None, :] // 64 == np.arange(2)[:, None]).astype(np.float32)

    def cols(wt, nt):     # [C_in, C_out] -> [128, nt*C_out]
        return np.ascontiguousarray(
            wt.reshape(nt, 128, C).transpose(1, 0, 2).reshape(128, nt * C))

    wpack = np.concatenate(
        [cols(qwT, CT), cols(kwT, KCT), cols(vwT, KCT), cols(pwT, CT)],
        axis=1).astype(BF)
    bpack = np.concatenate(
        [qb_eff.reshape(CT, 128).T, kb_eff.reshape(CT, 128).T,
         pb_eff.reshape(CT, 128).T, gnw8, gnb8], axis=1).astype(np.float32)
    ctxp = np.ascontiguousarray(
        context.reshape(B, LT, 128, CTX).transpose(2, 0, 1, 3)
        .reshape(128, B * LT * CTX)).astype(np.float32)

    shared = {
        "wpack": wpack,
        "bpack": np.ascontiguousarray(bpack),
        "ctx": ctxp,
        "selp": selp.astype(BF),
        "rmask": rmask,
        "bc2": bc2,
    }
    xr = x.reshape(B, C, S)
    in_maps = []
    for i in range(NCORES):
        xsh = np.ascontiguousarray(xr[:, :, i * s_loc:(i + 1) * s_loc])
        m = dict(shared)
        m["x"] = xsh.reshape(B, CT, 128, s_loc).astype(BF)
        in_maps.append(m)
    return in_maps


def _install_prof_shim():
    """Register the NTFF profile hook that this container's antenv lacks."""
    import types
    import antenv

    if "antenv.axon_hooks" not in sys.modules:
        mod = types.ModuleType("antenv.axon_hooks")
        mod._hook = None
        mod.set_axon_ntff_profile_hook = lambda h: setattr(mod, "_hook", h)
        mod.get_axon_ntff_profile_hook = lambda: mod._hook
        sys.modules["antenv.axon_hooks"] = mod
        antenv.axon_hooks = mod
    sys.path.insert(0, "/root/.axon_site")
    from trn_agent_boot.trn_boot import _ntff_profile_via_ctypes
    from antenv.axon_hooks import set_axon_ntff_profile_hook

    hook = _ntff_profile_via_ctypes("/opt/axon/libaxon_pjrt.so")
    assert hook is not None
    set_axon_ntff_profile_hook(hook)
    from concourse import bass_utils as bu
    bu.upload_artifacts = lambda tmpdir: tmpdir


def kernel(x, context, gn_w, gn_b, ln_w, ln_b, q_w, q_b, k_w, k_b,
           v_w, v_b, proj_w, proj_b):
    import os
    s_loc = S // NCORES
    st = 512
    in_maps = prep_inputs(x, context, gn_w, gn_b, ln_w, ln_b, q_w, q_b,
                          k_w, k_b, v_w, v_b, proj_w, proj_b, s_loc)
    nc = bacc.Bacc("TRN2", target_bir_lowering=False, debug=False,
                   num_devices=NCORES)
    build(nc, s_loc, st)
    nc.compile()
    trace = bool(os.environ.get("KPROF"))
    if trace:
        try:
            _install_prof_shim()
        except Exception as e:
            print(f"profiling shim unavailable ({e}); running untraced")
            trace = False
    try:
        res = run_bass_kernel_spmd(nc, in_maps, list(range(NCORES)),
                                   trace=trace,
                                   tmpdir=os.environ.get("KPROF_DIR"))
    except Exception:
        if not trace:
            raise
        print("traced run failed; retrying untraced")
        res = run_bass_kernel_spmd(nc, in_maps, list(range(NCORES)))
    if trace and res.exec_time_ns is not None:
        print(f"HW exec time: {res.exec_time_ns} ns")
    nst = s_loc // st
    out = np.empty((B, C, S), np.float32)
    for i in range(NCORES):
        r = res.results[i]["out"].astype(np.float32)
        r = r.reshape(B, nst, 128, CT, st)
        r = r.transpose(0, 3, 2, 1, 4).reshape(B, C, s_loc)
        out[:, :, i * s_loc:(i + 1) * s_loc] = r
    return out.reshape(B, C, D, H, W)


# revision 49
# speedup vs baseline: 1.0230x; 1.0230x over previous
"""CrossAttentionBlock3D on 8 TRN2 NeuronCores — sequence-parallel Bass kernel.

Sharding: the 32768 spatial tokens are split 8x4096 across cores. GroupNorm
statistics are the only cross-core dependency (one 64-float AllGather).
Everything else (LN, K/V projections over the tiny context, Q/attention/proj
for the local tokens) is computed locally; context-side work is replicated.

v2: GroupNorm folded into effective Q weights (no h materialization), x kept
resident in SBUF, interleaved emission to keep the PE warm (q of iter i+1 and
proj of iter i-1 fill the softmax-exp gaps), z head-pairs row-tiled on the PE
array, batched rowsum DMA + exp(-ln(x)) reciprocal, single activation table
set, bf16 output.
"""
import sys

sys.path.insert(0, "/opt/trn_rl_repo")

import numpy as np
import ml_dtypes

from concourse import bass, bacc, tile, mybir, masks
from concourse.bass_utils import run_bass_kernel_spmd

F32 = mybir.dt.float32
BF16 = mybir.dt.bfloat16
BF = ml_dtypes.bfloat16
AF = mybir.ActivationFunctionType
ALU = mybir.AluOpType

B, C, D, H, W = 2, 512, 32, 32, 32
S = D * H * W              # 32768
L, CTX = 256, 768
NH, HD, G = 8, 64, 8
EPS = 1e-5
NCORES = 8
CT, OT = 4, 4              # channel tiles (C = 4*128)
KCT = 6                    # ctx channel tiles (CTX = 6*128)
LT = 2                     # L = 2*128
VP = 80                    # padded per-(head,chunk) v stride: 64 v + 1 ones + pad


def build(nc, s_loc, st):
    """Emit the per-core Tile program. s_loc: local tokens; st: S-tile."""
    nst = s_loc // st
    n_gn = (C // G) * S    # global elems per (b, g) group

    WCOLS = 2 * CT * C + 2 * KCT * C          # qw | kw | vw | pw packed
    BCOLS = 3 * CT + 2 * B * CT               # qb | kb | pb | gnw8 | gnb8
    x_in = nc.dram_tensor("x", [B, CT, 128, s_loc], BF16, kind="ExternalInput")
    ctx_in = nc.dram_tensor("ctx", [128, B * LT * CTX], F32, kind="ExternalInput")
    w_in = nc.dram_tensor("wpack", [128, WCOLS], BF16, kind="ExternalInput")
    b_in = nc.dram_tensor("bpack", [128, BCOLS], F32, kind="ExternalInput")
    sel_in = nc.dram_tensor("selp", [8, 4 * 128], BF16, kind="ExternalInput")
    rm_in = nc.dram_tensor("rmask", [32, 4], F32, kind="ExternalInput")
    bc2_in = nc.dram_tensor("bc2", [2, 128], F32, kind="ExternalInput")
    out_ext = nc.dram_tensor("out", [B, nst, 128, CT * st], F32,
                             kind="ExternalOutput")

    from contextlib import ExitStack
    with tile.TileContext(nc) as tc, ExitStack() as es:
        wp = es.enter_context(tc.tile_pool(name="wp", bufs=1))
        dram = es.enter_context(tc.tile_pool(name="dram", bufs=1, space="DRAM"))

        # ---- persistent SBUF tensors ----
        w_all = wp.tile([128, WCOLS], BF16, tag="wpack")
        b_all = wp.tile([128, BCOLS], F32, tag="bpack")
        qw_t = w_all[:, 0:CT * C]
        kw_t = w_all[:, CT * C:CT * C + KCT * C]
        vw_t = w_all[:, CT * C + KCT * C:CT * C + 2 * KCT * C]
        pw_t = w_all[:, CT * C + 2 * KCT * C:WCOLS]
        qb_t = b_all[:, 0:CT]
        kb_t = b_all[:, CT:2 * CT]
        pb_t = b_all[:, 2 * CT:3 * CT]
        gnw_t = b_all[:, 3 * CT:3 * CT + B * CT]
        gnb_t = b_all[:, 3 * CT + B * CT:BCOLS]

        qw_eff = wp.tile([128, B * CT * C], BF16, tag="qw_eff")
        qb_tot = wp.tile([128, B * CT], F32, tag="qb_tot")
        x_all = wp.tile([128, B * CT * s_loc], BF16, tag="x_all")
        kT_all = wp.tile([128, B * CT * L], BF16, tag="kT")
        VW = NH * (HD + 1)
        v_all = wp.tile([128, B * LT * VW], BF16, tag="v")
        sel_t = wp.tile([8, 4 * 128], BF16, tag="selp")
        ident = wp.tile([128, 128], BF16, tag="ident")
        stats_s = wp.tile([128, 16], F32, tag="stats")
        a_pc = wp.tile([128, B * CT], F32, tag="a_pc")
        bias_pc = wp.tile([128, B * CT], BF16, tag="bias_pc")

        def xs(b, ct, lo=0, size=None):
            base = (b * CT + ct) * s_loc + lo
            return x_all[:, base:base + (size if size is not None else s_loc)]

        # ---- setup ----
        with tc.tile_pool(name="setup", bufs=1) as sp, \
             tc.tile_pool(name="setup_ps", bufs=2, space="PSUM") as spp:
            rm_t = sp.tile([32, 4], F32, tag="rm")
            bc2_t = sp.tile([2, 128], F32, tag="bc2")
            masks.make_identity(nc, ident[:])
            ctxf = sp.tile([128, B * LT * CTX], F32, tag="ctxf")
            eps2 = sp.tile([2, 1], F32, tag="eps2")
            nc.vector.memset(eps2[:], EPS)
            eps128 = sp.tile([128, 1], F32, tag="eps128")
            nc.vector.memset(eps128[:], EPS)

            # x loads (2 DMA queues) + GN partial stats
            # (sum in stats cols 0:8, sumsq in 8:16)
            sinkS = sp.tile([128, s_loc], BF16, tag="sinkS")

            def load_stats(b):
                for ct in range(CT):
                    col = b * CT + ct
                    nc.gpsimd.dma_start(xs(b, ct), x_in[b, ct]) if b == 0 \
                        else nc.sync.dma_start(xs(b, ct), x_in[b, ct])
                    nc.vector.tensor_reduce(
                        stats_s[:, col:col + 1], xs(b, ct),
                        mybir.AxisListType.X, ALU.add)
                    nc.scalar.activation(
                        sinkS[:], xs(b, ct), AF.Square,
                        accum_out=stats_s[:, 8 + col:9 + col])

            # x on the gpsimd queue (b0) so the sync queue serves ctx/weights
            # immediately; the ctx/K/V path below keeps the PE busy during
            # the x loads and the GN AllGather round-trip.
            nc.sync.dma_start(ctxf[:], ctx_in[:])
            nc.sync.dma_start(w_all[:], w_in[:])
            nc.sync.dma_start(b_all[:], b_in[:])
            nc.sync.dma_start(sel_t[:], sel_in[:])
            nc.sync.dma_start(rm_t[:], rm_in[:])
            nc.sync.dma_start(bc2_t[:], bc2_in[:])

            mask2 = sp.tile([128, 2], F32, tag="mask2")
            nc.vector.memset(mask2[:, :], 0.0)
            nc.vector.memset(mask2[0:64, 0:1], 1.0)
            nc.vector.memset(mask2[64:128, 1:2], 1.0)

            # per-batch: AllGather partial stats, derive qw_eff/qb_tot
            def ag_dispatch(b):
                cs = slice(b * CT, (b + 1) * CT)
                st_p = spp.tile([4, 4], F32, tag="st_p", bufs=1,
                                name=f"st_p_{b}")
                nc.tensor.matmul(st_p[:, 0:2], stats_s[:, b * CT:(b + 1) * CT],
                                 mask2[:], start=True, stop=True)
                nc.tensor.matmul(st_p[:, 2:4],
                                 stats_s[:, 8 + b * CT:8 + (b + 1) * CT],
                                 mask2[:], start=True, stop=True)
                red_s = sp.tile([4, 4], F32, tag="red_s", bufs=2,
                                name=f"red_s_{b}")
                nc.vector.tensor_copy(red_s[:], st_p[:])

                cc_in = dram.tile([4, 4], F32, tag="cc_in", bufs=2,
                                  name=f"cc_in_{b}")
                cc_ag = dram.tile([32, 4], F32, tag="cc_ag", bufs=2,
                                  name=f"cc_ag_{b}")
                nc.gpsimd.dma_start(cc_in[:], red_s[:])
                nc.gpsimd.collective_compute(
                    "AllGather", mybir.AluOpType.bypass,
                    replica_groups=[list(range(NCORES))],
                    ins=[cc_in.opt()], outs=[cc_ag.opt()])
                ag_s = sp.tile([32, 4], F32, tag="ag_s", bufs=2,
                               name=f"ag_s_{b}")
                nc.gpsimd.dma_start(ag_s[:], cc_ag[:])
                return ag_s

            def ag_post(b, ag_s):
                cs = slice(b * CT, (b + 1) * CT)
                ms_p = spp.tile([2, 8], F32, tag="ms_p", bufs=1,
                                name=f"ms_p_{b}")
                nc.tensor.matmul(ms_p[:, 0:4], ag_s[:, 0:2], rm_t[:],
                                 start=True, stop=True)
                nc.tensor.matmul(ms_p[:, 4:8], ag_s[:, 2:4], rm_t[:],
                                 start=True, stop=True)

                # mu | rstd  into one [2, 8] sbuf tile for one broadcast MM
                mr8 = sp.tile([2, 8], F32, tag="mr8", bufs=2, name=f"mr8_{b}")
                var8 = sp.tile([2, 4], F32, tag="var8", bufs=2,
                               name=f"var8_{b}")
                sd8 = sp.tile([2, 4], F32, tag="sd8", bufs=2, name=f"sd8_{b}")
                nc.vector.tensor_scalar_mul(mr8[:, 0:4], ms_p[:, 0:4],
                                            1.0 / n_gn)
                nc.vector.tensor_scalar_mul(var8[:], ms_p[:, 4:8], 1.0 / n_gn)
                # var = E[x^2] - mu^2
                tmu2 = sp.tile([2, 4], F32, tag="tmu2", bufs=2,
                               name=f"tmu2_{b}")
                nc.vector.tensor_mul(tmu2[:], mr8[:, 0:4], mr8[:, 0:4])
                nc.vector.tensor_sub(var8[:], var8[:], tmu2[:])
                # rstd = 1 / sqrt(var + eps)
                nc.scalar.activation(sd8[:], var8[:], AF.Sqrt, bias=eps2[:])
                nc.vector.reciprocal(mr8[:, 4:8], sd8[:])

                mr_d = dram.tile([2, 8], F32, tag="mr_d", bufs=2,
                                 name=f"mr_d_{b}")
                nc.gpsimd.dma_start(mr_d[:], mr8[:])
                br_s = sp.tile([128, 8], F32, tag="br_s", bufs=2,
                               name=f"br_s_{b}")
                nc.gpsimd.dma_start(
                    br_s[:], mr_d[:, :].unsqueeze(1).broadcast_to((2, 64, 8)))
                # a = rstd * gn_w ; bias = gn_b - mu * a   (per channel)
                nc.vector.tensor_mul(a_pc[:, cs], br_s[:, 4:8], gnw_t[:, cs])
                tmp_bc = sp.tile([128, 4], F32, tag="tmp_bc", bufs=2,
                                 name=f"tmp_bc_{b}")
                nc.vector.tensor_mul(tmp_bc[:], br_s[:, 0:4], a_pc[:, cs])
                nc.vector.tensor_sub(bias_pc[:, cs], gnb_t[:, cs], tmp_bc[:])

                # qw_eff[b] = qw * a  (per input-channel scale)
                for ct in range(CT):
                    col = b * CT + ct
                    nc.vector.tensor_scalar(
                        qw_eff[:, col * C:(col + 1) * C],
                        qw_t[:, ct * C:(ct + 1) * C],
                        a_pc[:, col:col + 1], None, ALU.mult)
                # qb_tot[b] = qw.T @ bias + qb   (bias zero-padded to N=4)
                bias_pad = sp.tile([128, CT * 4], BF16, tag="bias_pad",
                                   bufs=2, name=f"bias_pad_{b}")
                nc.vector.memset(bias_pad[:], 0.0)
                for ct in range(CT):
                    nc.vector.tensor_copy(
                        bias_pad[:, ct * 4:ct * 4 + 1],
                        bias_pc[:, b * CT + ct:b * CT + ct + 1])
                for ot in range(OT):
                    qe_p = spp.tile([128, 4], F32, tag="qe_p", bufs=1,
                                    name=f"qe_p_{b}_{ot}")
                    for ct in range(CT):
                        nc.tensor.matmul(
                            qe_p[:],
                            qw_t[:, ct * C + ot * 128:ct * C + (ot + 1) * 128],
                            bias_pad[:, ct * 4:(ct + 1) * 4],
                            start=(ct == 0), stop=(ct == CT - 1))
                    nc.vector.tensor_add(qb_tot[:, b * CT + ot:b * CT + ot + 1],
                                         qe_p[:, 0:1], qb_t[:, ot:ot + 1])

            # ---- context path: LN + transpose + K/V ----
            ctxn = sp.tile([128, B * LT * CTX], BF16, tag="ctxn")
            sinkC = sp.tile([128, CTX], F32, tag="sinkC")
            for b in range(B):
                for lt in range(LT):
                    cv = ctxf[:, (b * LT + lt) * CTX:(b * LT + lt + 1) * CTX]
                    cs1 = sp.tile([128, 1], F32, tag="cs1", bufs=2)
                    cs2 = sp.tile([128, 1], F32, tag="cs2", bufs=2)
                    nc.vector.tensor_reduce(cs1[:], cv, mybir.AxisListType.X,
                                            ALU.add)
                    nc.scalar.activation(sinkC[:], cv, AF.Square,
                                         accum_out=cs2[:])
                    cmu = sp.tile([128, 1], F32, tag="cmu", bufs=2)
                    cex2 = sp.tile([128, 1], F32, tag="cex2", bufs=2)
                    cvar = sp.tile([128, 1], F32, tag="cvar", bufs=2)
                    csd = sp.tile([128, 1], F32, tag="csd", bufs=2)
                    crstd = sp.tile([128, 1], F32, tag="crstd", bufs=2)
                    cnm = sp.tile([128, 1], F32, tag="cnm", bufs=2)
                    nc.vector.tensor_scalar_mul(cmu[:], cs1[:], 1.0 / CTX)
                    nc.vector.tensor_scalar_mul(cex2[:], cs2[:], 1.0 / CTX)
                    nc.vector.scalar_tensor_tensor(
                        cvar[:], cmu[:], -1.0, cmu[:],
                        ALU.mult, ALU.mult)
                    nc.vector.tensor_add(cvar[:], cvar[:], cex2[:])
                    nc.scalar.activation(csd[:], cvar[:], AF.Sqrt,
                                         bias=eps128[:])
                    nc.vector.reciprocal(crstd[:], csd[:])
                    nc.vector.scalar_tensor_tensor(
                        cnm[:], cmu[:], -1.0, crstd[:],
                        ALU.mult, ALU.mult)
                    nc.vector.tensor_scalar(
                        ctxn[:, (b * LT + lt) * CTX:(b * LT + lt + 1) * CTX],
                        cv, crstd[:], cnm[:],
                        ALU.mult, ALU.add)

            # transpose ctxn -> ctxT_all  [128ctx, L] per (b, kct)
            ctxT_all = sp.tile([128, B * KCT * L], BF16, tag="ctxT")
            for b in range(B):
                for lt in range(LT):
                    for ct in range(KCT):
                        tp_p = spp.tile([128, 128], BF16, tag="tp_p")
                        nc.tensor.transpose(
                            tp_p[:],
                            ctxn[:, (b * LT + lt) * CTX + ct * 128:
                                 (b * LT + lt) * CTX + (ct + 1) * 128],
                            ident[:])
                        nc.scalar.copy(
                            ctxT_all[:, (b * KCT + ct) * L + lt * 128:
                                     (b * KCT + ct) * L + (lt + 1) * 128],
                            tp_p[:])

            # kT[b, ot] [128, L]
            for b in range(B):
                for ot in range(OT):
                    k_p = spp.tile([128, L], F32, tag="k_p", bufs=1)
                    for ct in range(KCT):
                        nc.tensor.matmul(
                            k_p[:],
                            kw_t[:, ct * C + ot * 128:ct * C + (ot + 1) * 128],
                            ctxT_all[:, (b * KCT + ct) * L:(b * KCT + ct + 1) * L],
                            start=(ct == 0), stop=(ct == KCT - 1))
                    nc.scalar.activation(
                        kT_all[:, (b * CT + ot) * L:(b * CT + ot + 1) * L],
                        k_p[:], AF.Identity, bias=kb_t[:, ot:ot + 1])

            # v'[b, lt] [128, NH*(HD+1)]  (per-head ones column appended)
            for b in range(B):
                for lt in range(LT):
                    v_p = spp.tile([128, C], F32, tag="v_p", bufs=1)
                    for ct in range(KCT):
                        nc.tensor.matmul(
                            v_p[:],
                            ctxT_all[:, (b * KCT + ct) * L + lt * 128:
                                     (b * KCT + ct) * L + (lt + 1) * 128],
                            vw_t[:, ct * C:(ct + 1) * C],
                            start=(ct == 0), stop=(ct == KCT - 1))
                    vs = v_all[:, (b * LT + lt) * VW:(b * LT + lt + 1) * VW]
                    nc.scalar.copy(
                        vs.rearrange("p (h e) -> p h e", e=HD + 1)[:, :, 0:HD],
                        v_p[:])
                    nc.vector.memset(
                        vs.rearrange("p (h e) -> p h e", e=HD + 1)
                        [:, :, HD:HD + 1], 1.0)

            # GN stats + AllGathers last in emission order: b1's stats
            # reduces fill b0's collective round-trip, and the PE spends
            # the wait on the ctx/K/V work above.
            load_stats(0)
            ag0 = ag_dispatch(0)
            load_stats(1)
            ag_post(0, ag0)
            ag1 = ag_dispatch(1)
            ag_post(1, ag1)

        # ---- main attention loop (software-pipelined) ----
        with tc.tile_pool(name="mp", bufs=2) as mp, \
             tc.tile_pool(name="op", bufs=3) as op, \
             tc.tile_pool(name="mm_ps", bufs=2, space="PSUM") as mmp, \
             tc.tile_pool(name="z_ps", bufs=1, space="PSUM") as zp, \
             tc.tile_pool(name="o_ps", bufs=1, space="PSUM") as opp, \
             tc.tile_pool(name="rs_dram", bufs=2, space="DRAM") as rsd:

            iters = [(b, sti) for b in range(B) for sti in range(nst)]
            NIT = len(iters)

            def emit_q(i):
                b, sti = iters[i]
                lo = sti * st
                q_s = mp.tile([128, CT * st], BF16, tag="q_s",
                              name=f"q_s_{i}")
                for ot in range(OT):
                    q_p = mmp.tile([128, st], F32, tag="mm_p",
                                   name=f"q_p_{i}_{ot}")
                    for ct in range(CT):
                        nc.tensor.matmul(
                            q_p[:],
                            qw_eff[:, (b * CT + ct) * C + ot * 128:
                                   (b * CT + ct) * C + (ot + 1) * 128],
                            xs(b, ct, lo, st),
                            start=(ct == 0), stop=(ct == CT - 1))
                    nc.scalar.activation(
                        q_s[:, ot * st:(ot + 1) * st], q_p[:],
                        AF.Identity,
                        bias=qb_tot[:, b * CT + ot:b * CT + ot + 1])
                return q_s

            def emit_z_pair(i, k, q_s):
                """scores+exp for heads (2k, 2k+1). The two heads' z MMs
                alternate base partition 0/64 back-to-back, so consecutive
                MMs land in disjoint PE row groups and overlap."""
                b, sti = iters[i]
                zts, ps = [], []
                for j in range(2):
                    zts.append(zp.tile([128, 2 * st], F32, tag="z_t", bufs=2,
                                       name=f"z_{i}_{2 * k + j}"))
                for c in range(LT):
                    for j in range(2):
                        po = j * 64
                        nc.tensor.matmul(
                            zts[j][:, c * st:(c + 1) * st],
                            kT_all[po:po + 64,
                                   (b * CT + k) * L + c * 128:
                                   (b * CT + k) * L + (c + 1) * 128],
                            q_s[po:po + 64, k * st:(k + 1) * st],
                            start=True, stop=True)
                for j in range(2):
                    p_t = mp.tile([128, 2 * st], BF16, tag="p_t", bufs=4,
                                  name=f"p_{i}_{2 * k + j}")
                    nc.scalar.activation(p_t[:], zts[j][:], AF.Exp)
                    ps.append(p_t)
                return ps

            def emit_o_head(i, hh, p_t, o_all):
                b, sti = iters[i]
                o_p = opp.tile([65, st], F32, tag="o_p", bufs=2,
                               name=f"o_{i}_{hh}")
                for c in range(LT):
                    vb = (b * LT + c) * VW + hh * (HD + 1)
                    nc.tensor.matmul(
                        o_p[:],
                        v_all[:, vb:vb + HD + 1],
                        p_t[:, c * st:(c + 1) * st],
                        start=(c == 0), stop=(c == LT - 1))
                dst = o_all[:, hh * st:(hh + 1) * st]
                if hh % 2 == 0:
                    nc.scalar.copy(dst, o_p[:])
                else:
                    nc.vector.tensor_copy(dst, o_p[:])

            def emit_rs(i, o_all):
                rs_d = rsd.tile([8, st], F32, tag="rs_d",
                                name=f"rs_d_{i}")
                for h in range(NH):
                    nc.gpsimd.dma_start(rs_d[h:h + 1, :],
                                        o_all[64:65, h * st:(h + 1) * st])
                rs8 = mp.tile([8, st], F32, tag="rs8", name=f"rs8_{i}")
                nc.gpsimd.dma_start(rs8[:], rs_d[:])
                rec_f = mp.tile([8, st], F32, tag="rec_f", name=f"recf_{i}")
                nc.vector.reciprocal_approx_fast(rec_f[:], rs8[:])
                rec = mp.tile([8, st], BF16, tag="rec", name=f"rec_{i}")
                nc.vector.tensor_copy(rec[:], rec_f[:])
                # broadcast rec to head-pair layout [128, 4*st] via DMA
                # (keeps the normalize chain off the PE and the PSUM pools)
                rec_d = rsd.tile([8, st], BF16, tag="rec_d",
                                 name=f"rec_d_{i}")
                nc.gpsimd.dma_start(rec_d[:], rec[:])
                rb = mp.tile([64, NH * st], BF16, tag="rec_bc",
                             name=f"rb_{i}")
                for hh in range(NH):
                    nc.sync.dma_start(
                        rb[:, hh * st:(hh + 1) * st],
                        rec_d[hh:hh + 1, :].unsqueeze(1)
                        .broadcast_to((1, 64, st)))
                return rb

            def emit_norm(i, o_all, rb):
                proj_rhs = mp.tile([128, CT * st], BF16, tag="proj_rhs",
                                   name=f"prhs_{i}")
                for ko in range(4):
                    for j in range(2):
                        hh = 2 * ko + j
                        po = j * 64
                        nc.vector.tensor_tensor(
                            proj_rhs[po:po + 64, ko * st:(ko + 1) * st],
                            o_all[0:64, hh * st:(hh + 1) * st],
                            rb[:, hh * st:(hh + 1) * st], ALU.mult)
                return proj_rhs

            def emit_proj(i, proj_rhs, half):
                b, sti = iters[i]
                lo = sti * st
                if half == 0:
                    emit_proj.out_j = op.tile([128, CT * st], F32,
                                              tag="out_j", name=f"out_{i}")
                out_j = emit_proj.out_j
                for ot in (0, 1) if half == 0 else (2, 3):
                    y_p = mmp.tile([128, st], F32, tag="mm_p",
                                   name=f"y_p_{i}_{ot}")
                    for ct in range(CT):
                        nc.tensor.matmul(
                            y_p[:],
                            pw_t[:, ct * C + ot * 128:ct * C + (ot + 1) * 128],
                            proj_rhs[:, ct * st:(ct + 1) * st],
                            start=(ct == 0), stop=(ct == CT - 1))
                    nc.vector.scalar_tensor_tensor(
                        out_j[:, ot * st:(ot + 1) * st], y_p[:],
                        pb_t[:, ot:ot + 1], xs(b, ot, lo, st),
                        ALU.add, ALU.add)
                if half == 1:
                    nc.sync.dma_start(out_ext[b, sti], out_j[:])

            # software-pipelined emission: iter i-1's rowsum roundtrip,
            # normalize and proj are sandwiched between iter i's per-head
            # z/o chains so the PE always has independent work during the
            # softmax exp waits (keeps HAM from re-throttling the PE).
            q_cur = emit_q(0)
            prev = None            # (i, o_all)
            for i in range(NIT):
                o_all = mp.tile([65, NH * st], F32, tag="o_all",
                                name=f"o_all_{i}")
                if prev is not None:
                    rec_p = emit_rs(prev[0], prev[1])
                for k in range(4):
                    ps = emit_z_pair(i, k, q_cur)
                    emit_o_head(i, 2 * k, ps[0], o_all)
                    emit_o_head(i, 2 * k + 1, ps[1], o_all)
                q_next = emit_q(i + 1) if i + 1 < NIT else None
                if prev is not None:
                    prhs = emit_norm(prev[0], prev[1], rec_p)
                    emit_proj(prev[0], prhs, 0)
                    emit_proj(prev[0], prhs, 1)
                prev = (i, o_all)
                q_cur = q_next
            rec_p = emit_rs(prev[0], prev[1])
            prhs = emit_norm(prev[0], prev[1], rec_p)
            emit_proj(prev[0], prhs, 0)
            emit_proj(prev[0], prhs, 1)
    return nc


def prep_inputs(x, context, gn_w, gn_b, ln_w, ln_b, q_w, q_b, k_w, k_b,
                v_w, v_b, proj_w, proj_b, s_loc):
    """Host-side shard + layout prep. Returns in_maps for the 8 cores."""
    scale = HD ** -0.5
    qwT = (q_w.astype(np.float64) * scale).T.astype(np.float32)
    kwT = (k_w.astype(np.float64) * ln_w.astype(np.float64)[None, :]).T.astype(np.float32)
    vwT = (v_w.astype(np.float64) * ln_w.astype(np.float64)[None, :]).T.astype(np.float32)
    pwT = proj_w.T.astype(np.float32)
    kb_eff = (k_b + ln_b @ k_w.T).astype(np.float32)
    vb_eff = (v_b + ln_b @ v_w.T).astype(np.float32)
    pb_eff = (proj_b + vb_eff @ proj_w.T).astype(np.float32)
    qb_eff = (q_b * scale).astype(np.float32)

    gnw8 = np.empty((128, B * CT), np.float32)
    gnb8 = np.empty((128, B * CT), np.float32)
    for b in range(B):
        for t in range(CT):
            gnw8[:, b * CT + t] = gn_w[t * 128:(t + 1) * 128]
            gnb8[:, b * CT + t] = gn_b[t * 128:(t + 1) * 128]

    # pair-select: selp[r, ko*128 + m] = (r == 2*ko + (m >= 64))
    selp = np.zeros((8, 4 * 128), np.float32)
    for ko in range(4):
        selp[2 * ko, ko * 128:ko * 128 + 64] = 1.0
        selp[2 * ko + 1, ko * 128 + 64:(ko + 1) * 128] = 1.0
    rmask = (np.arange(32)[:, None] % 4 == np.arange(4)[None, :]).astype(np.float32)
    bc2 = (np.arange(128)[None, :] // 64 == np.arange(2)[:, None]).astype(np.float32)

    def cols(wt, nt):     # [C_in, C_out] -> [128, nt*C_out]
        return np.ascontiguousarray(
            wt.reshape(nt, 128, C).transpose(1, 0, 2).reshape(128, nt * C))

    wpack = np.concatenate(
        [cols(qwT, CT), cols(kwT, KCT), cols(vwT, KCT), cols(pwT, CT)],
        axis=1).astype(BF)
    bpack = np.concatenate(
        [qb_eff.reshape(CT, 128).T, kb_eff.reshape(CT, 128).T,
         pb_eff.reshape(CT, 128).T, gnw8, gnb8], axis=1).astype(np.float32)
    ctxp = np.ascontiguousarray(
        context.reshape(B, LT, 128, CTX).transpose(2, 0, 1, 3)
        .reshape(128, B * LT * CTX)).astype(np.float32)

    shared = {
        "wpack": wpack,
        "bpack": np.ascontiguousarray(bpack),
        "ctx": ctxp,
        "selp": selp.astype(BF),
        "rmask": rmask,
        "bc2": bc2,
    }
    xr = x.reshape(B, C, S)
    in_maps = []
    for i in range(NCORES):
        xsh = np.ascontiguousarray(xr[:, :, i * s_loc:(i + 1) * s_loc])
        m = dict(shared)
        m["x"] = xsh.reshape(B, CT, 128, s_loc).astype(BF)
        in_maps.append(m)
    return in_maps


def _install_prof_shim():
    """Register the NTFF profile hook that this container's antenv lacks."""
    import types
    import antenv

    if "antenv.axon_hooks" not in sys.modules:
        mod = types.ModuleType("antenv.axon_hooks")
        mod._hook = None
        mod.set_axon_ntff_profile_hook = lambda h: setattr(mod, "_hook", h)
        mod.get_axon_ntff_profile_hook = lambda: mod._hook
        sys.modules["antenv.axon_hooks"] = mod
        antenv.axon_hooks = mod
    sys.path.insert(0, "/root/.axon_site")
    from trn_agent_boot.trn_boot import _ntff_profile_via_ctypes
    from antenv.axon_hooks import set_axon_ntff_profile_hook

    hook = _ntff_profile_via_ctypes("/opt/axon/libaxon_pjrt.so")
    assert hook is not None
    set_axon_ntff_profile_hook(hook)
    from concourse import bass_utils as bu
    bu.upload_artifacts = lambda tmpdir: tmpdir


def kernel(x, context, gn_w, gn_b, ln_w, ln_b, q_w, q_b, k_w, k_b,
           v_w, v_b, proj_w, proj_b):
    import os
    s_loc = S // NCORES
    st = 512
    in_maps = prep_inputs(x, context, gn_w, gn_b, ln_w, ln_b, q_w, q_b,
                          k_w, k_b, v_w, v_b, proj_w, proj_b, s_loc)
    nc = bacc.Bacc("TRN2", target_bir_lowering=False, debug=False,
                   num_devices=NCORES)
    build(nc, s_loc, st)
    nc.compile()
    trace = bool(os.environ.get("KPROF"))
    if trace:
        try:
            _install_prof_shim()
        except Exception as e:
            print(f"profiling shim unavailable ({e}); running untraced")
            trace = False
    try:
        res = run_bass_kernel_spmd(nc, in_maps, list(range(NCORES)),
                                   trace=trace,
                                   tmpdir=os.environ.get("KPROF_DIR"))
    except Exception:
        if not trace:
            raise
        print("traced run failed; retrying untraced")
        res = run_bass_kernel_spmd(nc, in_maps, list(range(NCORES)))
    if trace and res.exec_time_ns is not None:
        print(f"HW exec time: {res.exec_time_ns} ns")
    nst = s_loc // st
    out = np.empty((B, C, S), np.float32)
    for i in range(NCORES):
        r = res.results[i]["out"].astype(np.float32)
        r = r.reshape(B, nst, 128, CT, st)
        r = r.transpose(0, 3, 2, 1, 4).reshape(B, C, s_loc)
        out[:, :, i * s_loc:(i + 1) * s_loc] = r
    return out.reshape(B, C, D, H, W)
